# revision 32
# baseline (speedup 1.0000x reference)
"""MultiHeadLinearAttention Trainium2 kernel (8 NeuronCores, SPMD), v3.

Sharding: core c handles batch b = c//2, head-group g = c%2 (4 of 8 heads,
i.e. feature slice F = [256g, 256g+256) of the 512 projection features).
Each core computes k/v/q projections restricted to its head-group, the
per-head linear-attention state over the full 8192-token sequence, and a
partial output  attn_F @ out_w[:, F].T.  The host sums the two partials per
batch and adds out_b.  No cross-core collectives.

Math per head h (matches the fp32 jax reference):
  proj(x)  = silu(x@w1.T + b1) * (x@w2.T + b2)
  phi(x)   = elu(x) + 1 = max(x+1, exp(min(x, 0)))
  kv[d,e]  = sum_s phi_k[s,d] v[s,e]        (64x64 per head)
  ksum[d]  = sum_s phi_k[s,d]
  attn[s,e]= (sum_d phi_q[s,d] kv[d,e]) / (sum_d phi_q[s,d] ksum[d])
  out      = attn @ out_w.T + out_b

Performance structure (v3, from hardware-trace iterations):
  * inputs/projection weights in fp16 (halves HBM traffic, 1 cyc/row mms)
  * projection BIASES folded into the inputs host-side: x' = x + c with
    c solving c @ [w1.T | 0.5 w2.T] = [b1 | 0.5 b2] (square well-conditioned
    system) -- eliminates all 128 bias matmuls and the q-side bias reads.
  * silu via tanh:  silu(x) = (tanh(x/2)+1) * x * 0.5, the 0.5 folded into
    w2/b2 host-side.  tanh/exp/relu/copy all live in one activation table
    ("exp_and_others") so the Act engine never reloads tables (v1 lost
    151us to Silu<->Exp thrash).
  * min(x,0) via Act relu:  exp(min(x,0)) = exp(-relu(-x)).
  * k and v projections share one 2-bank PSUM tile -> Act/DVE post-ops are
    batched [P,2,256] (half the instruction overheads).
  * denominators: one [4,512] PSUM accumulation per chunk, copied to SBUF,
    inverted by a 3-op bit-trick Newton reciprocal (exact reciprocal is
    ~6 cyc/elem; custom-DVE fused ops don't compile on this walrus).  The
    Chebyshev c0 constant is folded into out_w host-side.
  * reciprocal rows broadcast to 128 partitions by a tiny fp32r matmul
    whose destination shares the PSUM bank of the (already consumed)
    denominator accumulator.
  * k/v state matmuls merged per pair: [128,258] dst (kv | ksum) at
    1 cyc/row, ones column preloaded in the vproj tile.
  * software pipelining: state matmuls lag 2 subtiles behind projections;
    phase 2 runs qproj(i) | attention(i-1, two half-chunks) | output(i-2)
    with output subtiles interleaved between the halves.
  * fp16 partial outputs (summed in fp32 on the host).
"""
import sys
sys.path.insert(0, '/opt/trn_rl_repo')

import numpy as np
import concourse.bass as bass
import concourse.mybir as mybir
import concourse.tile as tile
from concourse.bass import ts, ds
from concourse.bass_utils import run_bass_kernel_spmd

F32 = mybir.dt.float32
F32R = mybir.dt.float32r
FP16 = mybir.dt.float16
I32 = mybir.dt.int32
AF = mybir.ActivationFunctionType
OP = mybir.AluOpType

B, S, D = 4, 8192, 512
NH, DK = 8, 64
FG = 256            # features per head-group (4 heads)
P = 128
CHUNK = 512         # tokens per streamed chunk
NCHUNK = S // CHUNK         # 16
SUBT = CHUNK // P           # 4 subtiles of 128 tokens per chunk
HC = CHUNK // 2             # attention half-chunk
LAG = 2                     # state-matmul lag (subtiles) behind projections

# Chebyshev-minimax seed constants for the bit-trick reciprocal
# (see concourse/dve_ops.py RECIP_APPROX_FAST_CONSTS)
RCP_C0 = -0.23549792
RCP_C1 = 2.0017324


def _split_waits(nc, limit=1):
    """walrus here rejects >1 embedded sync-wait per instruction; move extras
    onto same-engine NoOps immediately before (program order preserves
    semantics)."""
    uid = 0
    for f in nc.m.functions:
        for blk in f.blocks:
            new = []
            for ins in blk.instructions:
                si = ins.sync_info
                if si is not None and si.on_wait is not None and len(si.on_wait) > limit:
                    waits = list(si.on_wait)
                    head, keep = waits[:-limit], waits[-limit:]
                    for w in head:
                        nop = mybir.InstNoOp(
                            name=f"wsplit_{uid}", ins=[], outs=[],
                            sync_info=mybir.SyncInfo(on_wait=[w], on_update=[]))
                        uid += 1
                        nop.engine = ins.engine
                        new.append(nop)
                    ins.sync_info = mybir.SyncInfo(
                        on_wait=keep, on_update=list(si.on_update or []))
                new.append(ins)
            blk.instructions = new


def build_nc():
    nc = bass.Bass()

    # --- DRAM I/O (per-core data supplied via in_maps) ---
    xkT = nc.dram_tensor("xkT", [D, S], FP16, kind="ExternalInput")
    xvT = nc.dram_tensor("xvT", [D, S], FP16, kind="ExternalInput")
    xqT = nc.dram_tensor("xqT", [D, S], FP16, kind="ExternalInput")
    wk12T = nc.dram_tensor("wk12T", [D, 2 * FG], FP16, kind="ExternalInput")
    wv12T = nc.dram_tensor("wv12T", [D, 2 * FG], FP16, kind="ExternalInput")
    wq12T = nc.dram_tensor("wq12T", [D, 2 * FG], FP16, kind="ExternalInput")
    woT = nc.dram_tensor("woT", [FG, D], F32R, kind="ExternalInput")
    bdz = nc.dram_tensor("bdz", [P, 2 * P], F32R, kind="ExternalInput")
    dkz = nc.dram_tensor("dkz", [P, 8], F32R, kind="ExternalInput")
    sel4 = nc.dram_tensor("sel4", [4, 2 * P], F32R, kind="ExternalInput")
    ones2 = nc.dram_tensor("ones2", [P, 2], F32R, kind="ExternalInput")
    out = nc.dram_tensor("out", [S, D], FP16, kind="ExternalOutput")

    xkT_r = xkT.rearrange("(ko p) t -> p ko t", p=P)   # [128, 4, 8192]
    xvT_r = xvT.rearrange("(ko p) t -> p ko t", p=P)
    xqT_r = xqT.rearrange("(ko p) t -> p ko t", p=P)
    wk12T_r = wk12T.rearrange("(ko p) o -> p ko o", p=P)   # [128, 4, 512]
    wv12T_r = wv12T.rearrange("(ko p) o -> p ko o", p=P)
    wq12T_r = wq12T.rearrange("(ko p) o -> p ko o", p=P)
    woT_r = woT.rearrange("(ko p) o -> p ko o", p=P)       # [128, 2, 512]
    out_r = out.rearrange("(n p) f -> p n f", p=P)         # [128, 64, 512]

    with tile.TileContext(nc) as tc:
        with tc.tile_pool(name="const", bufs=1) as cpool:
            # Resident weights / constants
            wk_sb = cpool.tile([P, 4, 2 * FG], FP16)
            wv_sb = cpool.tile([P, 4, 2 * FG], FP16)
            wq_sb = cpool.tile([P, 4, 2 * FG], FP16)
            wo_sb = cpool.tile([P, 2, D], F32R)

            # Per-head-pair numerator/denominator lhsT built at phase boundary
            bd_sb = cpool.tile([P, 2, P], F32R)      # blockdiag kv per pair
            dk_sb = cpool.tile([P, 2, 4], F32R)      # zero-padded ksum cols
            sel4_sb = cpool.tile([4, 2, P], F32R)    # reciprocal row -> 64-blk

            # k/v projection output tiles: [:,0,:FG] kproj, [:,1,:FG] vproj,
            # [:,1,FG:FG+2] ones (fused ksum column for the state matmul)
            kvp_tiles = [cpool.tile([P, 2, FG + 2], F32R, name=f"kvp{i}")
                         for i in range(LAG + 2)]
            for kvp in kvp_tiles:
                nc.sync.dma_start(kvp[:, 1, FG:FG + 2], ones2[:])

            # ---------------- Phase 1: k/v projections + state ----------------
            ctx_iop2 = tc.tile_pool(name="p2_io", bufs=2)
            iop2 = ctx_iop2.__enter__()
            with tc.tile_pool(name="p1_io", bufs=3) as iop, \
                 tc.tile_pool(name="p1_sb", bufs=3) as sbp, \
                 tc.tile_pool(name="p1_phik", bufs=LAG + 2) as phpool, \
                 tc.tile_pool(name="p1_ps", bufs=3, space="PSUM") as psp, \
                 tc.tile_pool(name="p1_st", bufs=1, space="PSUM") as stp:

                state = [stp.tile([P, FG + 2], F32, name=f"st{p}", tag=f"st{p}")
                         for p in range(2)]

                pending = []  # deferred state matmuls: (phik, kvp)
                n_state = [0]

                def flush_state(last=False):
                    phik, kvp = pending.pop(0)
                    first = (n_state[0] == 0)
                    n_state[0] += 1
                    for p in range(2):
                        nc.tensor.matmul(state[p][:], phik[:, ts(p, P)],
                                         kvp[:, 1, :],
                                         start=first, stop=last)

                for c in range(NCHUNK):
                    kT_c = iop.tile([P, 4, CHUNK], FP16, tag="kT")
                    vT_c = iop.tile([P, 4, CHUNK], FP16, tag="vT")
                    nc.sync.dma_start(kT_c[:], xkT_r[:, :, ds(c * CHUNK, CHUNK)])
                    nc.sync.dma_start(vT_c[:], xvT_r[:, :, ds(c * CHUNK, CHUNK)])
                    if c == 0:
                        nc.sync.dma_start(wk_sb[:], wk12T_r[:])
                        nc.sync.dma_start(wv_sb[:], wv12T_r[:])
                    if c == 2:
                        # phase-2 constants: early enough to overlap phase 1,
                        # late enough not to delay the first k/v tiles
                        nc.sync.dma_start(wq_sb[:], wq12T_r[:])
                        nc.sync.dma_start(wo_sb[:], woT_r[:])
                        nc.sync.dma_start(
                            sel4_sb[:], sel4.rearrange("k (g m) -> k g m", m=P))
                    for s in range(SUBT):
                        tok = ds(s * P, P)
                        kv2 = psp.tile([P, 2, 2 * FG], F32, tag="kv")
                        for ki in range(4):
                            nc.tensor.matmul(kv2[:, 0, :], kT_c[:, ki, tok],
                                             wk_sb[:, ki, :],
                                             start=(ki == 0), stop=(ki == 3))
                        for ki in range(4):
                            nc.tensor.matmul(kv2[:, 1, :], vT_c[:, ki, tok],
                                             wv_sb[:, ki, :],
                                             start=(ki == 0), stop=(ki == 3))
                        # deferred state matmuls from LAG subtiles ago keep the
                        # PE busy while this subtile's phi chain is in flight
                        if len(pending) > LAG:
                            flush_state()
                        # silu via tanh for k and v in one batched op each
                        tkv = sbp.tile([P, 2, FG], F32, tag="tkv")
                        nc.scalar.activation(tkv[:], kv2[:, :, 0:FG], AF.Tanh,
                                             scale=0.5)
                        ukv = sbp.tile([P, 2, FG], F32, tag="ukv")
                        nc.vector.scalar_tensor_tensor(
                            ukv[:], tkv[:], 1.0, kv2[:, :, 0:FG], OP.add, OP.mult)
                        kvp = kvp_tiles[(c * SUBT + s) % len(kvp_tiles)]
                        nc.vector.tensor_tensor(kvp[:, :, 0:FG], ukv[:],
                                                kv2[:, :, FG:], OP.mult)
                        kview = kvp[:, 0, 0:FG].bitcast(F32)
                        # exp(min(x,0)) = exp(-relu(-x)) on the Act engine
                        rk = sbp.tile([P, FG], F32, tag="rk")
                        nc.scalar.activation(rk[:], kview, AF.Relu, scale=-1.0)
                        exk = sbp.tile([P, FG], F32, tag="exk")
                        nc.scalar.activation(exk[:], rk[:], AF.Exp, scale=-1.0)
                        phik = phpool.tile([P, FG], F32R, tag="phik")
                        nc.vector.scalar_tensor_tensor(
                            phik[:], kview, 1.0, exk[:], OP.add, OP.max)
                        pending.append((phik, kvp))
                while pending:
                    flush_state(last=(len(pending) == 1))

                # --- phase boundary: build bd (blockdiag kv) and dk (ksum cols)
                nc.sync.dma_start(bd_sb[:], bdz.rearrange("p (g m) -> p g m", m=P))
                nc.sync.dma_start(dk_sb[:], dkz.rearrange("p (g m) -> p g m", m=4))
                for p in range(2):
                    st = state[p]
                    nc.vector.tensor_copy(bd_sb[0:64, p, 0:64],
                                          st[0:64, ds(p * P, 64)])
                    nc.vector.tensor_copy(bd_sb[64:P, p, 64:P],
                                          st[64:P, ds(p * P + 64, 64)])
                    nc.vector.tensor_copy(dk_sb[0:64, p, ds(2 * p, 1)],
                                          st[0:64, ds(FG, 1)])
                    nc.vector.tensor_copy(dk_sb[64:P, p, ds(2 * p + 1, 1)],
                                          st[64:P, ds(FG + 1, 1)])

            # ---------------- Phase 2: q proj | attention | output pipeline --
            with tc.tile_pool(name="p2_sb", bufs=3) as sbp2, \
                 tc.tile_pool(name="p2_phiq", bufs=4) as phq, \
                 tc.tile_pool(name="p2_att", bufs=2) as attp, \
                 tc.tile_pool(name="p2_rcp", bufs=2) as rpool, \
                 tc.tile_pool(name="p2_ob", bufs=2) as obp, \
                 tc.tile_pool(name="p2_ps_q", bufs=3, space="PSUM") as psq, \
                 tc.tile_pool(name="p2_ps_rb", bufs=1, space="PSUM") as psd, \
                 tc.tile_pool(name="p2_ps_num", bufs=2, space="PSUM") as psn, \
                 tc.tile_pool(name="p2_ps_o", bufs=2, space="PSUM") as pso:

                phiq = {}   # chunk -> [m0, m1] F32R tiles
                att = {}    # chunk -> [p0, p1] F32R tiles
                rbqs = {}   # chunk -> dn/rb shared PSUM bank
                rcps = {}   # chunk -> [4, CHUNK] F32R reciprocal rows
                obs = {}    # chunk -> output staging tile

                def stage_a(c):  # q projection + phi_q for chunk c
                    qT_c = iop2.tile([P, 4, CHUNK], FP16, tag="qT")
                    nc.sync.dma_start(qT_c[:], xqT_r[:, :, ds(c * CHUNK, CHUNK)])
                    phiq[c] = []
                    for m in range(2):
                        ps1 = psq.tile([P, CHUNK], F32, tag="qp")
                        for ki in range(4):
                            nc.tensor.matmul(ps1[:], wq_sb[:, ki, ds(P * m, P)],
                                             qT_c[:, ki, :],
                                             start=(ki == 0), stop=(ki == 3))
                        ps2 = psq.tile([P, CHUNK], F32, tag="qp")
                        for ki in range(4):
                            nc.tensor.matmul(ps2[:], wq_sb[:, ki, ds(FG + P * m, P)],
                                             qT_c[:, ki, :],
                                             start=(ki == 0), stop=(ki == 3))
                        t = sbp2.tile([P, CHUNK], F32, tag="t")
                        nc.scalar.activation(t[:], ps1[:], AF.Tanh, scale=0.5)
                        u = sbp2.tile([P, CHUNK], F32, tag="u")
                        nc.vector.scalar_tensor_tensor(
                            u[:], t[:], 1.0, ps1[:], OP.add, OP.mult)
                        qp = sbp2.tile([P, CHUNK], F32, tag="qp_s")
                        nc.vector.tensor_tensor(qp[:], ps2[:], u[:], OP.mult)
                        rq = sbp2.tile([P, CHUNK], F32, tag="rq")
                        nc.scalar.activation(rq[:], qp[:], AF.Relu, scale=-1.0)
                        exq = sbp2.tile([P, CHUNK], F32, tag="exq")
                        nc.scalar.activation(exq[:], rq[:], AF.Exp, scale=-1.0)
                        phm = phq.tile([P, CHUNK], F32R, tag="phiq")
                        nc.vector.scalar_tensor_tensor(
                            phm[:], qp[:], 1.0, exq[:], OP.add, OP.max)
                        phiq[c].append(phm)

                def stage_b(c, h):  # denom + numer + attn, half-chunk h of c
                    tok = ds(h * HC, HC)
                    if h == 0:
                        att[c] = [attp.tile([P, CHUNK], F32R, name=f"att{p}",
                                            tag=f"att{p}") for p in range(2)]
                        # full-chunk denominator accumulation in the rb bank:
                        # the SBUF copy consumes it before the rb matmuls
                        # overwrite the bank.
                        rbqs[c] = psd.tile([P, 2, HC], F32, name="dnrb",
                                           tag="dnrb")
                        dn = rbqs[c].rearrange("p a b -> p (a b)")[0:4, :]
                        nc.tensor.matmul(dn, dk_sb[:, 0, :], phiq[c][0][:],
                                         start=True, stop=False,
                                         skip_group_check=True)
                        nc.tensor.matmul(dn, dk_sb[:, 1, :], phiq[c][1][:],
                                         start=False, stop=True,
                                         skip_group_check=True)
                        # 3-op bit-trick reciprocal (one Newton pass, ~0.25%):
                        #   n = bitcast(~dn); y' = ((n*-c0)*dn + c1) * n
                        # runs SBUF-only after one Act copy; the c0 factor is
                        # folded into out_w host-side.
                        dnsb = rpool.tile([4, CHUNK], F32, tag="dnsb")
                        nc.scalar.copy(dnsb[:], dn)
                        nb = rpool.tile([4, CHUNK], I32, tag="nb")
                        nc.vector.tensor_scalar(nb[:], dnsb[:].bitcast(I32),
                                                -1, None, OP.bitwise_xor)
                        nf = nb[:].bitcast(F32)
                        tt_ = rpool.tile([4, CHUNK], F32, tag="tt")
                        nc.vector.scalar_tensor_tensor(
                            tt_[:], nf, -RCP_C0, dnsb[:], OP.mult, OP.mult)
                        rcpr = rpool.tile([4, CHUNK], F32R, tag="rcpr")
                        nc.vector.scalar_tensor_tensor(
                            rcpr[:], tt_[:], RCP_C1, nf, OP.add, OP.mult)
                        rcps[c] = rcpr
                    rbq = rbqs[c]
                    # numerators first: they depend only on phi_q, so the PE
                    # isn't blocked behind the rb matmuls' reciprocal chain
                    numb = psn.tile([P, 2, HC], F32, tag="num")
                    for p in range(2):
                        nc.tensor.matmul(numb[:, p, :], bd_sb[:, p, :],
                                         phiq[c][p][:, tok],
                                         start=True, stop=True,
                                         skip_group_check=True)
                    # broadcast the 4 reciprocal rows onto 128 partitions
                    for p in range(2):
                        nc.tensor.matmul(rbq[:, p, :], sel4_sb[:, p, :],
                                         rcps[c][:, tok],
                                         start=(p == 0), stop=(p == 1),
                                         skip_group_check=True)
                    # DVE may read only one PSUM operand: stage rb to SBUF
                    rbs = rpool.tile([P, 2, HC], F32, tag="rbs")
                    if h == 0:
                        nc.scalar.copy(rbs[:], rbq[:])
                    else:
                        nc.vector.tensor_copy(rbs[:], rbq[:])
                    for p in range(2):
                        nc.vector.tensor_tensor(att[c][p][:, tok],
                                                numb[:, p, :], rbs[:, p, :],
                                                OP.mult)
                    if h == 1:
                        del phiq[c], rbqs[c], rcps[c]

                def stage_c(c, subs):  # output matmuls + store for chunk c
                    if subs[0] == 0:
                        obs[c] = obp.tile([P, SUBT, D], FP16, name="ob", tag="ob")
                    ob = obs[c]
                    for s in subs:
                        po = pso.tile([P, D], F32, tag="po")
                        nc.tensor.matmul(po[:], att[c][0][:, ts(s, P)], wo_sb[:, 0, :],
                                         start=True, stop=False)
                        nc.tensor.matmul(po[:], att[c][1][:, ts(s, P)], wo_sb[:, 1, :],
                                         start=False, stop=True)
                        if s != 3:
                            nc.scalar.copy(ob[:, s, :], po[:])
                        else:
                            nc.vector.tensor_copy(ob[:, s, :], po[:])
                    if subs[-1] == SUBT - 1:
                        nc.sync.dma_start(out_r[:, ds(c * SUBT, SUBT), :], ob[:])
                        del att[c]
                        del obs[c]

                for i in range(NCHUNK + 2):
                    if i < NCHUNK:
                        stage_a(i)
                    if 0 <= i - 1 < NCHUNK:
                        stage_b(i - 1, 0)
                    if 0 <= i - 2 < NCHUNK:
                        stage_c(i - 2, [0, 1])
                    if 0 <= i - 1 < NCHUNK:
                        stage_b(i - 1, 1)
                    if 0 <= i - 2 < NCHUNK:
                        stage_c(i - 2, [2, 3])

            ctx_iop2.__exit__(None, None, None)

    _split_waits(nc)
    return nc


_NC_CACHE = None


def _get_nc():
    global _NC_CACHE
    if _NC_CACHE is None:
        _NC_CACHE = build_nc()
    return _NC_CACHE


def _prep_in_maps(inputs):
    return _build_in_maps(
        inputs["query"], inputs["key"], inputs["value"],
        inputs["q_w1"], inputs["q_w2"], inputs["k_w1"], inputs["k_w2"],
        inputs["v_w1"], inputs["v_w2"], inputs["out_w"],
        inputs["q_b1"], inputs["q_b2"], inputs["k_b1"], inputs["k_b2"],
        inputs["v_b1"], inputs["v_b2"])


def _build_in_maps(query, key, value,
                   q_w1, q_w2, k_w1, k_w2, v_w1, v_w2, out_w,
                   q_b1, q_b2, k_b1, k_b2, v_b1, v_b2):
    query = np.asarray(query, dtype=np.float32)
    key = np.asarray(key, dtype=np.float32)
    value = np.asarray(value, dtype=np.float32)

    bdz = np.zeros((P, 2 * P), np.float32)
    dkz = np.zeros((P, 8), np.float32)
    sel4 = np.zeros((4, 2, P), np.float32)
    for p in range(2):
        sel4[2 * p, p, 0:64] = 1.0
        sel4[2 * p + 1, p, 64:P] = 1.0
    sel4 = sel4.reshape(4, 2 * P)

    def w12(w1, w2, Fs):
        # [512, 512] = [w1.T | 0.5*w2.T] restricted to feature slice Fs
        return np.ascontiguousarray(np.concatenate(
            [np.asarray(w1)[Fs].T, 0.5 * np.asarray(w2)[Fs].T],
            axis=1)).astype(np.float16)

    def bias_shift(w1, w2, b1, b2, Fs):
        # c with c @ [w1.T | 0.5 w2.T] = [b1 | 0.5 b2]: shifting x by c
        # bakes the projection biases into the matmul.
        A = np.concatenate([np.asarray(w1)[Fs], 0.5 * np.asarray(w2)[Fs]],
                           axis=0).astype(np.float64)
        bcat = np.concatenate([np.asarray(b1)[Fs], 0.5 * np.asarray(b2)[Fs]]
                              ).astype(np.float64)
        return np.linalg.solve(A, bcat).astype(np.float32)

    in_maps = []
    for c in range(8):
        b, g = c // 2, c % 2
        Fs = slice(FG * g, FG * (g + 1))
        ck = bias_shift(k_w1, k_w2, k_b1, k_b2, Fs)
        cv = bias_shift(v_w1, v_w2, v_b1, v_b2, Fs)
        cq = bias_shift(q_w1, q_w2, q_b1, q_b2, Fs)
        in_maps.append({
            "xkT": np.ascontiguousarray((key[b] + ck).T).astype(np.float16),
            "xvT": np.ascontiguousarray((value[b] + cv).T).astype(np.float16),
            "xqT": np.ascontiguousarray((query[b] + cq).T).astype(np.float16),
            "wk12T": w12(k_w1, k_w2, Fs),
            "wv12T": w12(v_w1, v_w2, Fs),
            "wq12T": w12(q_w1, q_w2, Fs),
            "woT": np.ascontiguousarray(
                RCP_C0 * np.asarray(out_w)[:, Fs].T).astype(np.float32),
            "bdz": bdz, "dkz": dkz, "sel4": sel4,
            "ones2": np.ones((P, 2), np.float32),
        })
    return in_maps


def kernel(query, key, value,
           q_w1, q_w2, k_w1, k_w2, v_w1, v_w2, out_w,
           q_b1, q_b2, k_b1, k_b2, v_b1, v_b2, out_b):
    in_maps = _build_in_maps(query, key, value,
                             q_w1, q_w2, k_w1, k_w2, v_w1, v_w2, out_w,
                             q_b1, q_b2, k_b1, k_b2, v_b1, v_b2)
    nc = _get_nc()
    res = run_bass_kernel_spmd(nc, in_maps, core_ids=list(range(8)))
    ob = np.asarray(out_b, dtype=np.float32)
    out = np.empty((B, S, D), np.float32)
    for b in range(B):
        out[b] = (np.asarray(res.results[2 * b]["out"], dtype=np.float32)
                  + np.asarray(res.results[2 * b + 1]["out"], dtype=np.float32)
                  + ob)
    return out


# revision 34
# speedup vs baseline: 1.0318x; 1.0318x over previous
"""MultiHeadLinearAttention Trainium2 kernel (8 NeuronCores, SPMD), v3.

Sharding: core c handles batch b = c//2, head-group g = c%2 (4 of 8 heads,
i.e. feature slice F = [256g, 256g+256) of the 512 projection features).
Each core computes k/v/q projections restricted to its head-group, the
per-head linear-attention state over the full 8192-token sequence, and a
partial output  attn_F @ out_w[:, F].T.  The host sums the two partials per
batch and adds out_b.  No cross-core collectives.

Math per head h (matches the fp32 jax reference):
  proj(x)  = silu(x@w1.T + b1) * (x@w2.T + b2)
  phi(x)   = elu(x) + 1 = max(x+1, exp(min(x, 0)))
  kv[d,e]  = sum_s phi_k[s,d] v[s,e]        (64x64 per head)
  ksum[d]  = sum_s phi_k[s,d]
  attn[s,e]= (sum_d phi_q[s,d] kv[d,e]) / (sum_d phi_q[s,d] ksum[d])
  out      = attn @ out_w.T + out_b

Performance structure (v3, from hardware-trace iterations):
  * inputs/projection weights in fp16 (halves HBM traffic, 1 cyc/row mms)
  * projection BIASES folded into the inputs host-side: x' = x + c with
    c solving c @ [w1.T | 0.5 w2.T] = [b1 | 0.5 b2] (square well-conditioned
    system) -- eliminates all 128 bias matmuls and the q-side bias reads.
  * silu via tanh:  silu(x) = (tanh(x/2)+1) * x * 0.5, the 0.5 folded into
    w2/b2 host-side.  tanh/exp/relu/copy all live in one activation table
    ("exp_and_others") so the Act engine never reloads tables (v1 lost
    151us to Silu<->Exp thrash).
  * min(x,0) via Act relu:  exp(min(x,0)) = exp(-relu(-x)).
  * k and v projections share one 2-bank PSUM tile -> Act/DVE post-ops are
    batched [P,2,256] (half the instruction overheads).
  * denominators: one [4,512] PSUM accumulation per chunk, copied to SBUF,
    inverted by a 3-op bit-trick Newton reciprocal (exact reciprocal is
    ~6 cyc/elem; custom-DVE fused ops don't compile on this walrus).  The
    Chebyshev c0 constant is folded into out_w host-side.
  * reciprocal rows broadcast to 128 partitions by a tiny fp32r matmul
    whose destination shares the PSUM bank of the (already consumed)
    denominator accumulator.
  * k/v state matmuls merged per pair: [128,258] dst (kv | ksum) at
    1 cyc/row, ones column preloaded in the vproj tile.
  * software pipelining: state matmuls lag 2 subtiles behind projections;
    phase 2 runs qproj(i) | attention(i-1, two half-chunks) | output(i-2)
    with output subtiles interleaved between the halves.
  * fp16 partial outputs (summed in fp32 on the host).
"""
import sys
sys.path.insert(0, '/opt/trn_rl_repo')

import numpy as np
import concourse.bass as bass
import concourse.mybir as mybir
import concourse.tile as tile
from concourse.bass import ts, ds
from concourse.bass_utils import run_bass_kernel_spmd

F32 = mybir.dt.float32
F32R = mybir.dt.float32r
FP16 = mybir.dt.float16
I32 = mybir.dt.int32
AF = mybir.ActivationFunctionType
OP = mybir.AluOpType

B, S, D = 4, 8192, 512
NH, DK = 8, 64
FG = 256            # features per head-group (4 heads)
P = 128
CHUNK = 512         # tokens per streamed chunk
NCHUNK = S // CHUNK         # 16
SUBT = CHUNK // P           # 4 subtiles of 128 tokens per chunk
HC = CHUNK // 2             # attention half-chunk
LAG = 2                     # state-matmul lag (subtiles) behind projections

# Chebyshev-minimax seed constants for the bit-trick reciprocal
# (see concourse/dve_ops.py RECIP_APPROX_FAST_CONSTS)
RCP_C0 = -0.23549792
RCP_C1 = 2.0017324


def _split_waits(nc, limit=1):
    """walrus here rejects >1 embedded sync-wait per instruction; move extras
    onto same-engine NoOps immediately before (program order preserves
    semantics)."""
    uid = 0
    for f in nc.m.functions:
        for blk in f.blocks:
            new = []
            for ins in blk.instructions:
                si = ins.sync_info
                if si is not None and si.on_wait is not None and len(si.on_wait) > limit:
                    waits = list(si.on_wait)
                    head, keep = waits[:-limit], waits[-limit:]
                    for w in head:
                        nop = mybir.InstNoOp(
                            name=f"wsplit_{uid}", ins=[], outs=[],
                            sync_info=mybir.SyncInfo(on_wait=[w], on_update=[]))
                        uid += 1
                        nop.engine = ins.engine
                        new.append(nop)
                    ins.sync_info = mybir.SyncInfo(
                        on_wait=keep, on_update=list(si.on_update or []))
                new.append(ins)
            blk.instructions = new


def build_nc():
    nc = bass.Bass()

    # --- DRAM I/O (per-core data supplied via in_maps) ---
    xkT = nc.dram_tensor("xkT", [D, S], FP16, kind="ExternalInput")
    xvT = nc.dram_tensor("xvT", [D, S], FP16, kind="ExternalInput")
    xqT = nc.dram_tensor("xqT", [D, S], FP16, kind="ExternalInput")
    wk12T = nc.dram_tensor("wk12T", [D, 2 * FG], FP16, kind="ExternalInput")
    wv12T = nc.dram_tensor("wv12T", [D, 2 * FG], FP16, kind="ExternalInput")
    wq12T = nc.dram_tensor("wq12T", [D, 2 * FG], FP16, kind="ExternalInput")
    woT = nc.dram_tensor("woT", [FG, D], F32R, kind="ExternalInput")
    bdz = nc.dram_tensor("bdz", [P, 2 * P], F32R, kind="ExternalInput")
    dkz = nc.dram_tensor("dkz", [P, 8], F32R, kind="ExternalInput")
    sel4 = nc.dram_tensor("sel4", [4, 2 * P], F32R, kind="ExternalInput")
    ones2 = nc.dram_tensor("ones2", [P, 2], F32R, kind="ExternalInput")
    out = nc.dram_tensor("out", [S, D], FP16, kind="ExternalOutput")

    xkT_r = xkT.rearrange("(ko p) t -> p ko t", p=P)   # [128, 4, 8192]
    xvT_r = xvT.rearrange("(ko p) t -> p ko t", p=P)
    xqT_r = xqT.rearrange("(ko p) t -> p ko t", p=P)
    wk12T_r = wk12T.rearrange("(ko p) o -> p ko o", p=P)   # [128, 4, 512]
    wv12T_r = wv12T.rearrange("(ko p) o -> p ko o", p=P)
    wq12T_r = wq12T.rearrange("(ko p) o -> p ko o", p=P)
    woT_r = woT.rearrange("(ko p) o -> p ko o", p=P)       # [128, 2, 512]
    out_r = out.rearrange("(n p) f -> p n f", p=P)         # [128, 64, 512]

    with tile.TileContext(nc) as tc:
        with tc.tile_pool(name="const", bufs=1) as cpool:
            # Resident weights / constants
            wk_sb = cpool.tile([P, 4, 2 * FG], FP16)
            wv_sb = cpool.tile([P, 4, 2 * FG], FP16)
            wq_sb = cpool.tile([P, 4, 2 * FG], FP16)
            wo_sb = cpool.tile([P, 2, D], F32R)

            # Per-head-pair numerator/denominator lhsT built at phase boundary
            bd_sb = cpool.tile([P, 2, P], F32R)      # blockdiag kv per pair
            dk_sb = cpool.tile([P, 2, 4], F32R)      # zero-padded ksum cols
            sel4_sb = cpool.tile([4, 2, P], F32R)    # reciprocal row -> 64-blk

            # k/v projection output tiles: [:,0,:FG] kproj, [:,1,:FG] vproj,
            # [:,1,FG:FG+2] ones (fused ksum column for the state matmul)
            kvp_tiles = [cpool.tile([P, 2, FG + 2], F32R, name=f"kvp{i}")
                         for i in range(LAG + 2)]
            for kvp in kvp_tiles:
                nc.sync.dma_start(kvp[:, 1, FG:FG + 2], ones2[:])

            # ---------------- Phase 1: k/v projections + state ----------------
            ctx_iop2 = tc.tile_pool(name="p2_io", bufs=2)
            iop2 = ctx_iop2.__enter__()
            with tc.tile_pool(name="p1_io", bufs=3) as iop, \
                 tc.tile_pool(name="p1_sb", bufs=3) as sbp, \
                 tc.tile_pool(name="p1_phik", bufs=LAG + 2) as phpool, \
                 tc.tile_pool(name="p1_ps", bufs=3, space="PSUM") as psp, \
                 tc.tile_pool(name="p1_st", bufs=1, space="PSUM") as stp:

                state = [stp.tile([P, FG + 2], F32, name=f"st{p}", tag=f"st{p}")
                         for p in range(2)]

                pending = []  # deferred state matmuls: (phik, kvp)
                n_state = [0]

                def flush_state(last=False):
                    phik, kvp = pending.pop(0)
                    first = (n_state[0] == 0)
                    n_state[0] += 1
                    for p in range(2):
                        nc.tensor.matmul(state[p][:], phik[:, ts(p, P)],
                                         kvp[:, 1, :],
                                         start=first, stop=last)

                for c in range(NCHUNK):
                    kT_c = iop.tile([P, 4, CHUNK], FP16, tag="kT")
                    vT_c = iop.tile([P, 4, CHUNK], FP16, tag="vT")
                    nc.sync.dma_start(kT_c[:], xkT_r[:, :, ds(c * CHUNK, CHUNK)])
                    nc.sync.dma_start(vT_c[:], xvT_r[:, :, ds(c * CHUNK, CHUNK)])
                    if c == 0:
                        nc.sync.dma_start(wk_sb[:], wk12T_r[:])
                        nc.sync.dma_start(wv_sb[:], wv12T_r[:])
                    if c == 2:
                        # phase-2 constants: early enough to overlap phase 1,
                        # late enough not to delay the first k/v tiles
                        nc.sync.dma_start(wq_sb[:], wq12T_r[:])
                        nc.sync.dma_start(wo_sb[:], woT_r[:])
                        nc.sync.dma_start(
                            sel4_sb[:], sel4.rearrange("k (g m) -> k g m", m=P))
                    for s in range(SUBT):
                        tok = ds(s * P, P)
                        kv2 = psp.tile([P, 2, 2 * FG], F32, tag="kv")
                        for ki in range(4):
                            nc.tensor.matmul(kv2[:, 0, :], kT_c[:, ki, tok],
                                             wk_sb[:, ki, :],
                                             start=(ki == 0), stop=(ki == 3))
                        for ki in range(4):
                            nc.tensor.matmul(kv2[:, 1, :], vT_c[:, ki, tok],
                                             wv_sb[:, ki, :],
                                             start=(ki == 0), stop=(ki == 3))
                        # deferred state matmuls from LAG subtiles ago keep the
                        # PE busy while this subtile's phi chain is in flight
                        if len(pending) > LAG:
                            flush_state()
                        # silu via tanh for k and v in one batched op each
                        tkv = sbp.tile([P, 2, FG], F32, tag="tkv")
                        nc.scalar.activation(tkv[:], kv2[:, :, 0:FG], AF.Tanh,
                                             scale=0.5)
                        ukv = sbp.tile([P, 2, FG], F32, tag="ukv")
                        nc.vector.scalar_tensor_tensor(
                            ukv[:], tkv[:], 1.0, kv2[:, :, 0:FG], OP.add, OP.mult)
                        kvp = kvp_tiles[(c * SUBT + s) % len(kvp_tiles)]
                        nc.vector.tensor_tensor(kvp[:, :, 0:FG], ukv[:],
                                                kv2[:, :, FG:], OP.mult)
                        kview = kvp[:, 0, 0:FG].bitcast(F32)
                        # exp(min(x,0)) = exp(-relu(-x)) on the Act engine
                        rk = sbp.tile([P, FG], F32, tag="rk")
                        nc.scalar.activation(rk[:], kview, AF.Relu, scale=-1.0)
                        exk = sbp.tile([P, FG], F32, tag="exk")
                        nc.scalar.activation(exk[:], rk[:], AF.Exp, scale=-1.0)
                        phik = phpool.tile([P, FG], F32R, tag="phik")
                        nc.vector.scalar_tensor_tensor(
                            phik[:], kview, 1.0, exk[:], OP.add, OP.max)
                        pending.append((phik, kvp))
                while pending:
                    flush_state(last=(len(pending) == 1))

                # --- phase boundary: build bd (blockdiag kv) and dk (ksum cols)
                nc.sync.dma_start(bd_sb[:], bdz.rearrange("p (g m) -> p g m", m=P))
                nc.sync.dma_start(dk_sb[:], dkz.rearrange("p (g m) -> p g m", m=4))
                for p in range(2):
                    st = state[p]
                    nc.vector.tensor_copy(bd_sb[0:64, p, 0:64],
                                          st[0:64, ds(p * P, 64)])
                    nc.vector.tensor_copy(bd_sb[64:P, p, 64:P],
                                          st[64:P, ds(p * P + 64, 64)])
                    nc.vector.tensor_copy(dk_sb[0:64, p, ds(2 * p, 1)],
                                          st[0:64, ds(FG, 1)])
                    nc.vector.tensor_copy(dk_sb[64:P, p, ds(2 * p + 1, 1)],
                                          st[64:P, ds(FG + 1, 1)])

            # ---------------- Phase 2: q proj | attention | output pipeline --
            with tc.tile_pool(name="p2_sb", bufs=3) as sbp2, \
                 tc.tile_pool(name="p2_phiq", bufs=4) as phq, \
                 tc.tile_pool(name="p2_att", bufs=2) as attp, \
                 tc.tile_pool(name="p2_rcp", bufs=2) as rpool, \
                 tc.tile_pool(name="p2_ob", bufs=2) as obp, \
                 tc.tile_pool(name="p2_ps_q", bufs=3, space="PSUM") as psq, \
                 tc.tile_pool(name="p2_ps_rb", bufs=1, space="PSUM") as psd, \
                 tc.tile_pool(name="p2_ps_num", bufs=2, space="PSUM") as psn, \
                 tc.tile_pool(name="p2_ps_o", bufs=2, space="PSUM") as pso:

                phiq = {}   # chunk -> [m0, m1] F32R tiles
                att = {}    # chunk -> [p0, p1] F32R tiles
                rbqs = {}   # chunk -> dn/rb shared PSUM bank
                rcps = {}   # chunk -> [4, CHUNK] F32R reciprocal rows
                obs = {}    # chunk -> output staging tile

                def stage_a(c):  # q projection + phi_q for chunk c
                    qT_c = iop2.tile([P, 4, CHUNK], FP16, tag="qT")
                    nc.sync.dma_start(qT_c[:], xqT_r[:, :, ds(c * CHUNK, CHUNK)])
                    phiq[c] = []
                    for m in range(2):
                        ps1 = psq.tile([P, CHUNK], F32, tag="qp")
                        for ki in range(4):
                            nc.tensor.matmul(ps1[:], wq_sb[:, ki, ds(P * m, P)],
                                             qT_c[:, ki, :],
                                             start=(ki == 0), stop=(ki == 3))
                        ps2 = psq.tile([P, CHUNK], F32, tag="qp")
                        for ki in range(4):
                            nc.tensor.matmul(ps2[:], wq_sb[:, ki, ds(FG + P * m, P)],
                                             qT_c[:, ki, :],
                                             start=(ki == 0), stop=(ki == 3))
                        t = sbp2.tile([P, CHUNK], F32, tag="t")
                        nc.scalar.activation(t[:], ps1[:], AF.Tanh, scale=0.5)
                        u = sbp2.tile([P, CHUNK], F32, tag="u")
                        nc.vector.scalar_tensor_tensor(
                            u[:], t[:], 1.0, ps1[:], OP.add, OP.mult)
                        qp = sbp2.tile([P, CHUNK], F32, tag="qp_s")
                        nc.vector.tensor_tensor(qp[:], ps2[:], u[:], OP.mult)
                        rq = sbp2.tile([P, CHUNK], F32, tag="rq")
                        nc.scalar.activation(rq[:], qp[:], AF.Relu, scale=-1.0)
                        exq = sbp2.tile([P, CHUNK], F32, tag="exq")
                        nc.scalar.activation(exq[:], rq[:], AF.Exp, scale=-1.0)
                        phm = phq.tile([P, CHUNK], F32R, tag="phiq")
                        nc.vector.scalar_tensor_tensor(
                            phm[:], qp[:], 1.0, exq[:], OP.add, OP.max)
                        phiq[c].append(phm)

                def stage_b(c, h):  # denom + numer + attn, half-chunk h of c
                    tok = ds(h * HC, HC)
                    if h == 0:
                        att[c] = [attp.tile([P, CHUNK], F32R, name=f"att{p}",
                                            tag=f"att{p}") for p in range(2)]
                        # full-chunk denominator accumulation in the rb bank:
                        # the SBUF copy consumes it before the rb matmuls
                        # overwrite the bank.
                        rbqs[c] = psd.tile([P, 2, HC], F32, name="dnrb",
                                           tag="dnrb")
                        dn = rbqs[c].rearrange("p a b -> p (a b)")[0:4, :]
                        nc.tensor.matmul(dn, dk_sb[:, 0, :], phiq[c][0][:],
                                         start=True, stop=False,
                                         skip_group_check=True)
                        nc.tensor.matmul(dn, dk_sb[:, 1, :], phiq[c][1][:],
                                         start=False, stop=True,
                                         skip_group_check=True)
                        # 3-op bit-trick reciprocal (one Newton pass, ~0.25%):
                        #   n = bitcast(~dn); y' = ((n*-c0)*dn + c1) * n
                        # runs SBUF-only after one Act copy; the c0 factor is
                        # folded into out_w host-side.
                        dnsb = rpool.tile([4, CHUNK], F32, tag="dnsb")
                        nc.scalar.copy(dnsb[:], dn)
                        nb = rpool.tile([4, CHUNK], I32, tag="nb")
                        nc.vector.tensor_scalar(nb[:], dnsb[:].bitcast(I32),
                                                -1, None, OP.bitwise_xor)
                        nf = nb[:].bitcast(F32)
                        tt_ = rpool.tile([4, CHUNK], F32, tag="tt")
                        nc.vector.scalar_tensor_tensor(
                            tt_[:], nf, -RCP_C0, dnsb[:], OP.mult, OP.mult)
                        rcpr = rpool.tile([4, CHUNK], F32R, tag="rcpr")
                        nc.vector.scalar_tensor_tensor(
                            rcpr[:], tt_[:], RCP_C1, nf, OP.add, OP.mult)
                        rcps[c] = rcpr
                    rbq = rbqs[c]
                    # numerators first: they depend only on phi_q, so the PE
                    # isn't blocked behind the rb matmuls' reciprocal chain
                    numb = psn.tile([P, 2, HC], F32, tag="num")
                    for p in range(2):
                        nc.tensor.matmul(numb[:, p, :], bd_sb[:, p, :],
                                         phiq[c][p][:, tok],
                                         start=True, stop=True,
                                         skip_group_check=True)
                    # broadcast the 4 reciprocal rows onto 128 partitions
                    for p in range(2):
                        nc.tensor.matmul(rbq[:, p, :], sel4_sb[:, p, :],
                                         rcps[c][:, tok],
                                         start=(p == 0), stop=(p == 1),
                                         skip_group_check=True)
                    # DVE may read only one PSUM operand: stage rb to SBUF
                    rbs = rpool.tile([P, 2, HC], F32, tag="rbs")
                    nc.scalar.copy(rbs[:], rbq[:])
                    for p in range(2):
                        nc.vector.tensor_tensor(att[c][p][:, tok],
                                                numb[:, p, :], rbs[:, p, :],
                                                OP.mult)
                    if h == 1:
                        del phiq[c], rbqs[c], rcps[c]

                def stage_c(c, subs):  # output matmuls + store for chunk c
                    if subs[0] == 0:
                        obs[c] = obp.tile([P, SUBT, D], FP16, name="ob", tag="ob")
                    ob = obs[c]
                    for s in subs:
                        po = pso.tile([P, D], F32, tag="po")
                        nc.tensor.matmul(po[:], att[c][0][:, ts(s, P)], wo_sb[:, 0, :],
                                         start=True, stop=False)
                        nc.tensor.matmul(po[:], att[c][1][:, ts(s, P)], wo_sb[:, 1, :],
                                         start=False, stop=True)
                        if s % 2 == 0:
                            nc.scalar.copy(ob[:, s, :], po[:])
                        else:
                            nc.vector.tensor_copy(ob[:, s, :], po[:])
                    if subs[-1] == SUBT - 1:
                        nc.sync.dma_start(out_r[:, ds(c * SUBT, SUBT), :], ob[:])
                        del att[c]
                        del obs[c]

                for i in range(NCHUNK + 2):
                    if i < NCHUNK:
                        stage_a(i)
                    if 0 <= i - 1 < NCHUNK:
                        stage_b(i - 1, 0)
                    if 0 <= i - 2 < NCHUNK:
                        stage_c(i - 2, [0, 1])
                    if 0 <= i - 1 < NCHUNK:
                        stage_b(i - 1, 1)
                    if 0 <= i - 2 < NCHUNK:
                        stage_c(i - 2, [2, 3])

            ctx_iop2.__exit__(None, None, None)

    _split_waits(nc)
    return nc


_NC_CACHE = None


def _get_nc():
    global _NC_CACHE
    if _NC_CACHE is None:
        _NC_CACHE = build_nc()
    return _NC_CACHE


def _prep_in_maps(inputs):
    return _build_in_maps(
        inputs["query"], inputs["key"], inputs["value"],
        inputs["q_w1"], inputs["q_w2"], inputs["k_w1"], inputs["k_w2"],
        inputs["v_w1"], inputs["v_w2"], inputs["out_w"],
        inputs["q_b1"], inputs["q_b2"], inputs["k_b1"], inputs["k_b2"],
        inputs["v_b1"], inputs["v_b2"])


def _build_in_maps(query, key, value,
                   q_w1, q_w2, k_w1, k_w2, v_w1, v_w2, out_w,
                   q_b1, q_b2, k_b1, k_b2, v_b1, v_b2):
    query = np.asarray(query, dtype=np.float32)
    key = np.asarray(key, dtype=np.float32)
    value = np.asarray(value, dtype=np.float32)

    bdz = np.zeros((P, 2 * P), np.float32)
    dkz = np.zeros((P, 8), np.float32)
    sel4 = np.zeros((4, 2, P), np.float32)
    for p in range(2):
        sel4[2 * p, p, 0:64] = 1.0
        sel4[2 * p + 1, p, 64:P] = 1.0
    sel4 = sel4.reshape(4, 2 * P)

    def w12(w1, w2, Fs):
        # [512, 512] = [w1.T | 0.5*w2.T] restricted to feature slice Fs
        return np.ascontiguousarray(np.concatenate(
            [np.asarray(w1)[Fs].T, 0.5 * np.asarray(w2)[Fs].T],
            axis=1)).astype(np.float16)

    def bias_shift(w1, w2, b1, b2, Fs):
        # c with c @ [w1.T | 0.5 w2.T] = [b1 | 0.5 b2]: shifting x by c
        # bakes the projection biases into the matmul.
        A = np.concatenate([np.asarray(w1)[Fs], 0.5 * np.asarray(w2)[Fs]],
                           axis=0).astype(np.float64)
        bcat = np.concatenate([np.asarray(b1)[Fs], 0.5 * np.asarray(b2)[Fs]]
                              ).astype(np.float64)
        return np.linalg.solve(A, bcat).astype(np.float32)

    in_maps = []
    for c in range(8):
        b, g = c // 2, c % 2
        Fs = slice(FG * g, FG * (g + 1))
        ck = bias_shift(k_w1, k_w2, k_b1, k_b2, Fs)
        cv = bias_shift(v_w1, v_w2, v_b1, v_b2, Fs)
        cq = bias_shift(q_w1, q_w2, q_b1, q_b2, Fs)
        in_maps.append({
            "xkT": np.ascontiguousarray((key[b] + ck).T).astype(np.float16),
            "xvT": np.ascontiguousarray((value[b] + cv).T).astype(np.float16),
            "xqT": np.ascontiguousarray((query[b] + cq).T).astype(np.float16),
            "wk12T": w12(k_w1, k_w2, Fs),
            "wv12T": w12(v_w1, v_w2, Fs),
            "wq12T": w12(q_w1, q_w2, Fs),
            "woT": np.ascontiguousarray(
                RCP_C0 * np.asarray(out_w)[:, Fs].T).astype(np.float32),
            "bdz": bdz, "dkz": dkz, "sel4": sel4,
            "ones2": np.ones((P, 2), np.float32),
        })
    return in_maps


def kernel(query, key, value,
           q_w1, q_w2, k_w1, k_w2, v_w1, v_w2, out_w,
           q_b1, q_b2, k_b1, k_b2, v_b1, v_b2, out_b):
    in_maps = _build_in_maps(query, key, value,
                             q_w1, q_w2, k_w1, k_w2, v_w1, v_w2, out_w,
                             q_b1, q_b2, k_b1, k_b2, v_b1, v_b2)
    nc = _get_nc()
    res = run_bass_kernel_spmd(nc, in_maps, core_ids=list(range(8)))
    ob = np.asarray(out_b, dtype=np.float32)
    out = np.empty((B, S, D), np.float32)
    for b in range(B):
        out[b] = (np.asarray(res.results[2 * b]["out"], dtype=np.float32)
                  + np.asarray(res.results[2 * b + 1]["out"], dtype=np.float32)
                  + ob)
    return out
